# revision 25
# baseline (speedup 1.0000x reference)
"""Box filter (radius 8, window 17, zero-padded edges) over dims 2,3 of a
[8, 32, 512, 512] f32 tensor, on 8 Trainium2 NeuronCores.

Decomposition (validated vs the jax reference, rel err ~1e-6):
  - The per-axis filter with clipped windows is exactly multiplication by a
    banded ones matrix B (B[i,k] = 1 iff |i-k| <= 8), i.e. Z = B @ X @ B.
  - Column (free-dim) filter: ONE fused DVE `tensor_tensor_scan` per row-tile
    computes the sliding-window sum directly via the recurrence
        state[t] = (x[t] + state[t-1]) - x[t-17]
    over a zero-padded buffer (17 zeros in front, 8 behind), so scan output
    position t holds the window ending at t; the window *centered* at c is
    position c+8, read as a simple offset view.
  - Row (partition-dim) filter: one PE matmul per 112-row output tile with a
    host-built banded lhsT (input tiles carry an 8-row halo on each side, so
    one K<=128 matmul covers the whole band).

Sharding: data-parallel over batch (dim 0) -> 8 cores, one batch each.
"""

import os
import sys

import numpy as np

for _p in ("/opt/trn_rl_repo", "/root/.axon_site/_ro/trn_rl_repo"):
    if os.path.isdir(_p) and _p not in sys.path:
        sys.path.append(_p)

import concourse.bass as bass
import concourse.tile as tile
from concourse import bacc, mybir
from concourse.bass_utils import run_bass_kernel_spmd

R = 8
PADF = 2 * R + 1  # front zero pad (window width)
PADB = R          # back zero pad
H = W = 512
CH = 32
NCORES = 8

# Row-tile specs: (row_start, n_rows_loaded, use_first_B, out_rows, out_start).
# Output tiles are 112 rows; input tiles carry the +-8 halo (clipped at the
# image edges), so a single matmul covers the full 17-row band.
SPECS = [
    (0, 120, True, 112, 0),
    (104, 128, False, 112, 112),
    (216, 128, False, 112, 224),
    (328, 128, False, 112, 336),
    (440, 72, False, 64, 448),
]

_CACHE = {}


def _banded():
    # Bl[k, m] = 1 iff the input row at tile partition k (image row
    # 112*t - 8 + k) is inside the window of output row m (image row 112*t+m):
    # |(m + 8) - k| <= 8  <=>  m <= k <= m + 16.
    k = np.arange(128)[:, None]
    m = np.arange(112)[None, :]
    bl = ((m <= k) & (k <= m + 16)).astype(np.float32)
    # First tile starts at image row 0 (no left halo): partition k = image
    # row k, band |k - m| <= 8 — which is bl shifted down 8 partitions.
    blf = bl[8:128].copy()
    return bl, blf


USE_F32R = os.environ.get("BOX_F32R", "0") == "1"


def _build_program():
    if "nc" in _CACHE:
        return _CACHE["nc"]
    # Bacc (not raw Bass): its compile() legalizes sync waits — TRN2 allows
    # at most 1 wait per instruction; excess waits become standalone
    # EventSemaphore instructions (and matmul waits move to ldweights).
    nc = bacc.Bacc(debug=False)
    # float32r (tf32) matmul operands run the PE at 1 cycle/row instead of
    # fp32's 2x half-speed passes; the walrus verifier requires fp32r
    # operands to be produced as fp32r, so the B constants and the scan
    # output use the dtype end-to-end (same 4-byte storage as fp32).
    mm_dt = mybir.dt.float32r if USE_F32R else mybir.dt.float32
    x = nc.dram_tensor("x", [CH, H, W], mybir.dt.float32, kind="ExternalInput")
    z = nc.dram_tensor("z", [CH, H, W], mybir.dt.float32, kind="ExternalOutput")
    bl = nc.dram_tensor("bl", [128, 112], mm_dt, kind="ExternalInput")
    blf = nc.dram_tensor("blf", [120, 112], mm_dt, kind="ExternalInput")
    xap, zap = x.ap(), z.ap()

    f32 = mybir.dt.float32
    XW = PADF + W + PADB  # 537

    # DMA trigger cost is ~650 ns FIXED per instruction (measured: 64- and
    # 128-descriptor transfers cost the same), so batch transfers: one 3-tile
    # load (768 KB, overlapping strided source AP) and one 4-tile store
    # (896 KB) per channel, plus small t=0 / t=4 edge transfers.
    #
    # All 5 row-tiles of a channel live in ONE [128, 5*537] buffer; the 25
    # zeros between adjacent blocks (8 back pad + 17 front pad) flush the
    # scan recurrence, so ONE scan instruction per channel covers all tiles.
    NBIG = 4
    NOBIG = 4
    XALL = 5 * XW  # 2685

    # Block order in the per-channel buffer: [t0][t4][t1][t2][t3], so the two
    # edge tiles share one scan and the three interior tiles share another
    # (the 25 zeros between blocks flush the recurrence at each seam).
    with tile.TileContext(nc) as tc:
        with (
            tc.tile_pool(name="consts", bufs=1) as cpool,
            tc.tile_pool(name="ub01", bufs=5) as upool01,
            tc.tile_pool(name="ub234", bufs=5) as upool234,
            tc.tile_pool(name="ob4", bufs=4) as o4pool,
            tc.tile_pool(name="psum", bufs=8, space="PSUM") as ppool,
        ):
            blt = cpool.tile([128, 112], mm_dt)
            blft = cpool.tile([120, 112], mm_dt)

            # Static ring; each buffer zeroed once (lazily, so channel 0's
            # loads start immediately) — loads only ever touch the data
            # columns of partitions [0, nr), so pads and unused partitions
            # stay zero for the whole kernel.
            xalls = [
                nc.alloc_sbuf_tensor(f"xall{i}", [128, XALL], f32).ap()
                for i in range(NBIG)
            ]
            obigs = [
                nc.alloc_sbuf_tensor(f"obig{i}", [112, 4, W], f32).ap()
                for i in range(NOBIG)
            ]

            for c in range(CH):
                xa = xalls[c % NBIG]
                og = obigs[c % NOBIG]
                if c < NBIG:
                    nc.vector.memset(xa[:, :], 0.0)

                # t=0 edge load: rows 0..119 -> block 0
                nc.sync.dma_start(
                    xa[0:120, PADF:PADF + W], xap[c, 0:120, :]
                )
                # t=4 edge load: rows 440..511 -> block 1
                nc.sync.dma_start(
                    xa[0:72, XW + PADF:XW + PADF + W],
                    xap[c, 440:512, :],
                )
                # batched t=1..3 load into blocks 2..4:
                # element (p, b, col) <- x[c, 104 + 112*b + p, col]
                src = bass.AP(
                    tensor=x,
                    offset=(c * H + 104) * W,
                    ap=[[W, 128], [112 * W, 3], [1, W]],
                )
                dst = bass.AP(
                    tensor=xa.tensor,
                    offset=xa.offset + 2 * XW + PADF,
                    ap=[[XALL, 128], [XW, 3], [1, W]],
                )
                nc.sync.dma_start(dst, src)
                if c == 0:
                    # const loads after the first channel's loads so they
                    # don't delay the pipeline head
                    nc.sync.dma_start(blt[:], bl.ap()[:, :])
                    nc.sync.dma_start(blft[:], blf.ap()[:, :])

                # scan 1: blocks 0..1 (edge tiles t=0 and t=4)
                u01 = upool01.tile([128, 2 * XW - PADF], mm_dt)
                nc.vector.tensor_tensor_scan(
                    out=u01[:, :],
                    data0=xa[:, PADF:2 * XW],
                    data1=xa[:, 0:2 * XW - PADF],
                    initial=0.0,
                    op0=mybir.AluOpType.add,
                    op1=mybir.AluOpType.subtract,
                )
                # scan 2: blocks 2..4 (interior tiles t=1..3)
                u234 = upool234.tile([128, 3 * XW - PADF], mm_dt)
                nc.vector.tensor_tensor_scan(
                    out=u234[:, :],
                    data0=xa[:, 2 * XW + PADF:XALL],
                    data1=xa[:, 2 * XW:XALL - PADF],
                    initial=0.0,
                    op0=mybir.AluOpType.add,
                    op1=mybir.AluOpType.subtract,
                )

                for (r0, nr, first, m_out, o0) in SPECS:
                    t = o0 // 112
                    if t == 0:
                        u = u01[0:nr, R:R + W]
                    elif t == 4:
                        u = u01[0:nr, XW + R:XW + R + W]
                    else:
                        u = u234[0:nr, (t - 1) * XW + R:(t - 1) * XW + R + W]
                    ps = ppool.tile([112, 512], f32)
                    lhsT = blft[0:nr, 0:m_out] if first else blt[0:nr, 0:m_out]
                    nc.tensor.matmul(
                        ps[0:m_out, :], lhsT, u, start=True, stop=True,
                    )
                    if t < 4:
                        nc.scalar.copy(og[:, t, :], ps[0:112, :])
                        if t == 3 and c < CH - 1:
                            # batched t=0..3 store; follows the copies on the
                            # scalar queue in program order (no extra waits)
                            nc.scalar.dma_start(
                                zap[c, 0:448, :].rearrange(
                                    "(t p) w -> p t w", p=112
                                ),
                                og[:, :, :],
                            )
                        elif c == CH - 1:
                            # last channel: store per tile so the final
                            # transfer is small (shorter kernel tail)
                            nc.scalar.dma_start(
                                zap[c, o0:o0 + 112, :], og[:, t, :]
                            )
                    else:
                        ob = o4pool.tile([64, 512], f32)
                        nc.scalar.copy(ob[0:64, :], ps[0:64, :])
                        nc.scalar.dma_start(zap[c, 448:512, :], ob[0:64, :])

    nc.compile()
    _CACHE["nc"] = nc
    return nc


def kernel(tensor: np.ndarray) -> np.ndarray:
    tensor = np.ascontiguousarray(np.asarray(tensor, dtype=np.float32))
    assert tensor.shape == (NCORES, CH, H, W)
    bl, blf = _banded()
    nc = _build_program()
    in_maps = [
        {"x": tensor[i], "bl": bl, "blf": blf} for i in range(NCORES)
    ]
    res = run_bass_kernel_spmd(nc, in_maps, core_ids=list(range(NCORES)))
    return np.stack([res.results[i]["z"] for i in range(NCORES)], axis=0)


# revision 29
# speedup vs baseline: 1.0203x; 1.0203x over previous
"""Box filter (radius 8, window 17, zero-padded edges) over dims 2,3 of a
[8, 32, 512, 512] f32 tensor, on 8 Trainium2 NeuronCores.

Decomposition (validated vs the jax reference, rel err ~1e-6):
  - The per-axis filter with clipped windows is exactly multiplication by a
    banded ones matrix B (B[i,k] = 1 iff |i-k| <= 8), i.e. Z = B @ X @ B.
  - Column (free-dim) filter: ONE fused DVE `tensor_tensor_scan` per row-tile
    computes the sliding-window sum directly via the recurrence
        state[t] = (x[t] + state[t-1]) - x[t-17]
    over a zero-padded buffer (17 zeros in front, 8 behind), so scan output
    position t holds the window ending at t; the window *centered* at c is
    position c+8, read as a simple offset view.
  - Row (partition-dim) filter: one PE matmul per 112-row output tile with a
    host-built banded lhsT (input tiles carry an 8-row halo on each side, so
    one K<=128 matmul covers the whole band).

Sharding: data-parallel over batch (dim 0) -> 8 cores, one batch each.
"""

import os
import sys

import numpy as np

for _p in ("/opt/trn_rl_repo", "/root/.axon_site/_ro/trn_rl_repo"):
    if os.path.isdir(_p) and _p not in sys.path:
        sys.path.append(_p)

import concourse.bass as bass
import concourse.tile as tile
from concourse import bacc, mybir
from concourse.bass_utils import run_bass_kernel_spmd

R = 8
PADF = 2 * R + 1  # front zero pad (window width)
PADB = R          # back zero pad
H = W = 512
CH = 32
NCORES = 8

# Row-tile specs: (row_start, n_rows_loaded, use_first_B, out_rows, out_start).
# Output tiles are 112 rows; input tiles carry the +-8 halo (clipped at the
# image edges), so a single matmul covers the full 17-row band.
SPECS = [
    (0, 120, True, 112, 0),
    (104, 128, False, 112, 112),
    (216, 128, False, 112, 224),
    (328, 128, False, 112, 336),
    (440, 72, False, 64, 448),
]

_CACHE = {}


def _banded():
    # Bl[k, m] = 1 iff the input row at tile partition k (image row
    # 112*t - 8 + k) is inside the window of output row m (image row 112*t+m):
    # |(m + 8) - k| <= 8  <=>  m <= k <= m + 16.
    k = np.arange(128)[:, None]
    m = np.arange(112)[None, :]
    bl = ((m <= k) & (k <= m + 16)).astype(np.float32)
    # First tile starts at image row 0 (no left halo): partition k = image
    # row k, band |k - m| <= 8 — which is bl shifted down 8 partitions.
    blf = bl[8:128].copy()
    return bl, blf


USE_F32R = os.environ.get("BOX_F32R", "0") == "1"


def _build_program():
    if "nc" in _CACHE:
        return _CACHE["nc"]
    # Bacc (not raw Bass): its compile() legalizes sync waits — TRN2 allows
    # at most 1 wait per instruction; excess waits become standalone
    # EventSemaphore instructions (and matmul waits move to ldweights).
    nc = bacc.Bacc(debug=False)
    # float32r (tf32) matmul operands run the PE at 1 cycle/row instead of
    # fp32's 2x half-speed passes; the walrus verifier requires fp32r
    # operands to be produced as fp32r, so the B constants and the scan
    # output use the dtype end-to-end (same 4-byte storage as fp32).
    mm_dt = mybir.dt.float32r if USE_F32R else mybir.dt.float32
    x = nc.dram_tensor("x", [CH, H, W], mybir.dt.float32, kind="ExternalInput")
    z = nc.dram_tensor("z", [CH, H, W], mybir.dt.float32, kind="ExternalOutput")
    bl = nc.dram_tensor("bl", [128, 112], mm_dt, kind="ExternalInput")
    blf = nc.dram_tensor("blf", [120, 112], mm_dt, kind="ExternalInput")
    xap, zap = x.ap(), z.ap()

    f32 = mybir.dt.float32
    XW = PADF + W + PADB  # 537

    # DMA trigger cost is ~650 ns FIXED per instruction (measured: 64- and
    # 128-descriptor transfers cost the same), so batch transfers: one 3-tile
    # load (768 KB, overlapping strided source AP) and one 4-tile store
    # (896 KB) per channel, plus small t=0 / t=4 edge transfers.
    #
    # All 5 row-tiles of a channel live in ONE [128, 5*537] buffer; the 25
    # zeros between adjacent blocks (8 back pad + 17 front pad) flush the
    # scan recurrence, so ONE scan instruction per channel covers all tiles.
    NBIG = 4
    NOBIG = 4
    XALL = 5 * XW  # 2685

    with tile.TileContext(nc) as tc:
        with (
            tc.tile_pool(name="consts", bufs=1) as cpool,
            tc.tile_pool(name="ubuf", bufs=10) as upool,
            tc.tile_pool(name="ob4", bufs=4) as o4pool,
            tc.tile_pool(name="psum", bufs=8, space="PSUM") as ppool,
        ):
            blt = cpool.tile([128, 112], mm_dt)
            blft = cpool.tile([120, 112], mm_dt)

            # Static ring; each buffer zeroed once, lazily, so channel 0's
            # loads issue immediately — loads only ever touch the data
            # columns of partitions [0, nr), so pads and unused partitions
            # stay zero for the whole kernel.
            xalls = [
                nc.alloc_sbuf_tensor(f"xall{i}", [128, XALL], f32).ap()
                for i in range(NBIG)
            ]
            nc.vector.memset(xalls[0][:, :], 0.0)
            obigs = [
                nc.alloc_sbuf_tensor(f"obig{i}", [112, 4, W], f32).ap()
                for i in range(NOBIG)
            ]

            for c in range(CH):
                xa = xalls[c % NBIG]
                og = obigs[c % NOBIG]

                # t=0 edge load: rows 0..119 -> block 0
                nc.sync.dma_start(
                    xa[0:120, PADF:PADF + W], xap[c, 0:120, :]
                )
                # batched t=1..3 load into blocks 1..3:
                # element (p, b, col) <- x[c, 104 + 112*b + p, col]
                src = bass.AP(
                    tensor=x,
                    offset=(c * H + 104) * W,
                    ap=[[W, 128], [112 * W, 3], [1, W]],
                )
                dst = bass.AP(
                    tensor=xa.tensor,
                    offset=xa.offset + XW + PADF,
                    ap=[[XALL, 128], [XW, 3], [1, W]],
                )
                nc.sync.dma_start(dst, src)
                # t=4 edge load: rows 440..511 -> block 4
                nc.sync.dma_start(
                    xa[0:72, 4 * XW + PADF:4 * XW + PADF + W],
                    xap[c, 440:512, :],
                )
                if c == 0:
                    # consts after channel 0's loads (first consumer is the
                    # first matmul, well past the pipeline head); remaining
                    # ring buffers zeroed here to overlap with c=0's DMAs
                    nc.sync.dma_start(blt[:], bl.ap()[:, :])
                    nc.sync.dma_start(blft[:], blf.ap()[:, :])
                    for xb in xalls[1:]:
                        nc.vector.memset(xb[:, :], 0.0)

                for (r0, nr, first, m_out, o0) in SPECS:
                    t = o0 // 112
                    # per-tile scan over this tile's block (fine-grained so
                    # the matmul/copy/store pipeline stays tightly packed)
                    base = t * XW
                    ub = upool.tile([128, W + PADB], mm_dt)
                    nc.vector.tensor_tensor_scan(
                        out=ub[0:nr, :],
                        data0=xa[0:nr, base + PADF:base + XW],
                        data1=xa[0:nr, base:base + W + PADB],
                        initial=0.0,
                        op0=mybir.AluOpType.add,
                        op1=mybir.AluOpType.subtract,
                    )
                    ps = ppool.tile([112, 512], f32)
                    lhsT = blft[0:nr, 0:m_out] if first else blt[0:nr, 0:m_out]
                    nc.tensor.matmul(
                        ps[0:m_out, :], lhsT, ub[0:nr, R:R + W],
                        start=True, stop=True,
                    )
                    if t < 4:
                        nc.scalar.copy(og[:, t, :], ps[0:112, :])
                        if t == 3 and c < CH - 1:
                            # batched t=0..3 store; follows the copies on the
                            # scalar queue in program order (no extra waits)
                            nc.scalar.dma_start(
                                zap[c, 0:448, :].rearrange(
                                    "(t p) w -> p t w", p=112
                                ),
                                og[:, :, :],
                            )
                        elif c == CH - 1:
                            # last channel: per-tile stores so the kernel
                            # tail ends on a small transfer
                            nc.scalar.dma_start(
                                zap[c, o0:o0 + 112, :], og[:, t, :]
                            )
                    else:
                        ob = o4pool.tile([64, 512], f32)
                        nc.scalar.copy(ob[0:64, :], ps[0:64, :])
                        nc.scalar.dma_start(zap[c, 448:512, :], ob[0:64, :])

    nc.compile()
    _CACHE["nc"] = nc
    return nc


def kernel(tensor: np.ndarray) -> np.ndarray:
    tensor = np.ascontiguousarray(np.asarray(tensor, dtype=np.float32))
    assert tensor.shape == (NCORES, CH, H, W)
    bl, blf = _banded()
    nc = _build_program()
    in_maps = [
        {"x": tensor[i], "bl": bl, "blf": blf} for i in range(NCORES)
    ]
    res = run_bass_kernel_spmd(nc, in_maps, core_ids=list(range(NCORES)))
    return np.stack([res.results[i]["z"] for i in range(NCORES)], axis=0)
